# revision 1
# baseline (speedup 1.0000x reference)
"""Bass/Tile TRN2 kernel for nn_FCPairedLayer (pairwise-feature MLP).

Math: the reference builds pair features v[b,i,j] (384 = 6 blocks of 64
channels), each block depending on a single (possibly shifted) row index:
  v = [x[i], x[j], m_u*x[i-1], m_u*x[j+1], m_d*x[i+1], m_d*x[j-1]]
with m_u(i,j) = [i>=1][j<=N-2], m_d(i,j) = [i<=N-2][j>=1].
Hence a = W1^T v + b1 = R[i] + C[j] away from the grid border, where
  R[i] = W1_0^T x[i] + W1_2^T x[i-1] + W1_4^T x[i+1]
  C[j] = W1_1^T x[j] + W1_3^T x[j+1] + W1_5^T x[j-1] + b1
and the border rows/cols (where a mask kills a whole term) use fixed-up
variants:
  col j=0:   R_left  = W1_0^T x[i] + W1_2^T x[i-1]        (no W1_4 term)
  col j=N-1: R_right = W1_0^T x[i] + W1_4^T x[i+1]        (no W1_2 term)
  row i=0:   C_top   = C without the W1_3 (x[j+1]) term
  row i=N-1: C_bot   = C without the W1_5 (x[j-1]) term
Then y = W2^T relu(a) + b2.

On-device per core (128 of the 1024 (b,i) rows):
  - tiny PE matmuls build R/C variants (256 hidden channels = 2 chunks
    of 128 partitions, positions along the free dim)
  - per output row: fused add+relu via DVE tensor_scalar(add,max) or
    ACT activation(Relu, bias=R[:,i]) on C tiles -> h[128, 510]
  - 256->1 dot: M=1 float32r matmuls into PSUM partition strips
    (tile_position=(0,32r), 4 rows per PSUM bank), ACT-copy to SBUF,
    DMA out.  Border columns j=0/511 and the 4 corners get their own
    small passes.
b2 is added on the host (single scalar).
"""

import numpy as np

import concourse.bacc as bacc
import concourse.bass as bass
import concourse.mybir as mybir
import concourse.tile as tile
from concourse import bass_utils

F32 = mybir.dt.float32
F32R = mybir.dt.float32r
BF16 = mybir.dt.float16
ALU = mybir.AluOpType
ACTF = mybir.ActivationFunctionType

B = 2
N = 512
CIN = 64
H = 256  # hidden; 2 chunks of 128
NCORES = 8
ROWS = 128  # (b,i) rows per core
NI = N - 2  # interior columns per row

# fraction of elementwise (add+relu) ops that go to DVE; rest to ACT
DVE_FRAC = 0.83
H_BUFS = 8
YPS_BUFS = 4
YST_BUFS = 3
COPY_ALT = 0  # 0: all ACT, 1: alternate DVE/ACT
# run the C-variant prep matmuls in float32r (4x faster, slight rounding)
PREP_F32R = True
# run the 256->1 dot matmuls in float32r (4x faster than f32)
DOT_F32R = True

LAST_RESULTS = None
_CACHED_NC = None


def _mm_cast(ap, enable=True):
    return ap


def _build_program(repeat=1):
    nc = bacc.Bacc("TRN2", target_bir_lowering=False, debug=False)

    xpqr = nc.dram_tensor("xpqr", [CIN, 3 * (N + 2)], F32R, kind="ExternalInput")
    wpack = nc.dram_tensor("wpack", [128, 132 + 3 * H], F32R, kind="ExternalInput")
    w2v = nc.dram_tensor("w2v", [128, 64], BF16, kind="ExternalInput")
    y = nc.dram_tensor("y", [ROWS, N], F32, kind="ExternalOutput")

    with tile.TileContext(nc) as tc:
        import contextlib

        with contextlib.ExitStack() as ctx:
          const = ctx.enter_context(tc.tile_pool(name="const", bufs=1))
          prep_ps = ctx.enter_context(
              tc.tile_pool(name="prep_ps", bufs=3, space="PSUM")
          )
          col_ps = ctx.enter_context(tc.tile_pool(name="col_ps", bufs=1, space="PSUM"))
          col_sb = ctx.enter_context(tc.tile_pool(name="col_sb", bufs=2))
          h_pool = ctx.enter_context(tc.tile_pool(name="h", bufs=H_BUFS))
          y_ps = ctx.enter_context(tc.tile_pool(name="y_ps", bufs=YPS_BUFS, space="PSUM"))
          y_st = ctx.enter_context(tc.tile_pool(name="y_st", bufs=YST_BUFS))
          for _rep in range(repeat):

            # ---- load inputs to SBUF (few big contiguous DMAs) ----
            # halo at partitions 0-63 (even W1 blocks), p/q/r at 64-127 (odd)
            wp_s = const.tile([128, 132 + 3 * H], F32R, name="wp_s", tag="wp_s")
            nc.sync.dma_start(wp_s[:, 0 : 132 + H], wpack.ap()[:, 0 : 132 + H])
            nc.sync.dma_start(wp_s[:, 132 + H :], wpack.ap()[:, 132 + H :])
            xh_s = wp_s[0:64, 0 : ROWS + 2]
            b1_s = wp_s[:, 130:132].bitcast(F32)
            xpqr_s = const.tile([128, 3 * (N + 2)], F32R, name="xpqr_s")
            nc.sync.dma_start(xpqr_s[64:128, 0 : N + 2], xpqr.ap()[:, 0 : N + 2])
            nc.sync.dma_start(
                xpqr_s[64:128, N + 2 :], xpqr.ap()[:, N + 2 :]
            )
            xTh_s = xh_s
            xTp_s = xpqr_s[64:128, 0 : N + 2]
            xTq_s = xpqr_s[64:128, N + 2 : 2 * (N + 2)]
            xTr_s = xpqr_s[64:128, 2 * (N + 2) : 3 * (N + 2)]

            class _W1B:
                def __getitem__(self, kh):
                    k, h = kh
                    p0 = 64 * (k % 2)
                    c0 = 132 + H * (k // 2) + 128 * h
                    return wp_s[p0 : p0 + 64, c0 : c0 + 128]

            w1b = _W1B()

            w2t = const.tile([128, 64], BF16, name="w2t", tag="w2t")
            nc.sync.dma_start(w2t[:], w2v.ap()[:])
            w2_s = {h: w2t[:, 32 * h : 32 * h + 32] for h in range(2)}



            # ---- prep: R variants, one segmented PSUM per chunk ----
            # psum segments [left | mid | right] (3 x 128 = 384 >= 256 keeps
            # float32r matmuls at 1 cyc/row); every term hits a contiguous
            # segment range:
            #   P0 (x[i])    -> all three    P2s (x[i-1]) -> left+mid
            #   P4s (x[i+1]) -> mid+right
            r_tiles = {}
            for h in range(2):
                ps_full = prep_ps.tile([128, N], F32, tag="prep", name=f"psr_{h}")
                nc.tensor.matmul(
                    ps_full[:, 0 : 3 * ROWS],
                    w1b[(0, h)],
                    xTh_s[:, 1 : 1 + ROWS].unsqueeze(1).broadcast_to((CIN, 3, ROWS)),
                    start=True,
                    stop=False,
                    skip_group_check=True,
                )
                nc.tensor.matmul(
                    ps_full[:, 0 : 2 * ROWS],
                    w1b[(2, h)],
                    xTh_s[:, 0:ROWS].unsqueeze(1).broadcast_to((CIN, 2, ROWS)),
                    start=False,
                    stop=False,
                    skip_group_check=True,
                )
                nc.tensor.matmul(
                    ps_full[:, ROWS : 3 * ROWS],
                    w1b[(4, h)],
                    xTh_s[:, 2 : 2 + ROWS].unsqueeze(1).broadcast_to((CIN, 2, ROWS)),
                    start=False,
                    stop=True,
                    skip_group_check=True,
                )
                rall = const.tile([128, 3 * ROWS], F32, name=f"R_all_{h}", tag=f"R_all_{h}")
                nc.vector.tensor_copy(rall[:], ps_full[:, 0 : 3 * ROWS])
                r_tiles[("left", h)] = rall[:, 0:ROWS]
                r_tiles[("mid", h)] = rall[:, ROWS : 2 * ROWS]
                r_tiles[("right", h)] = rall[:, 2 * ROWS : 3 * ROWS]

            # ---- prep: C variants [128, N] per chunk ----
            # C_mid = W1_1^T x[j] + W1_3^T x[j+1] + W1_5^T x[j-1] + b1
            # C_top: W1_3 term built from xTq (zeroed when core owns row 0)
            # C_bot: W1_5 term built from xTr (zeroed when core owns row N-1)
            c_tiles = {}
            c_specs = {
                "mid": [(1, xTp_s, 1), (3, xTp_s, 2), (5, xTp_s, 0)],
                "top": [(1, xTp_s, 1), (3, xTq_s, 2), (5, xTp_s, 0)],
                "bot": [(1, xTp_s, 1), (3, xTp_s, 2), (5, xTr_s, 0)],
            }
            for vname, terms in c_specs.items():
                for h in range(2):
                    ps = prep_ps.tile([128, N], F32, tag="prep")
                    for t_i, (k, src, off) in enumerate(terms):
                        nc.tensor.matmul(
                            ps[:],
                            _mm_cast(w1b[(k, h)], PREP_F32R),
                            _mm_cast(src[:, off : off + N], PREP_F32R),
                            start=(t_i == 0),
                            stop=(t_i == len(terms) - 1),
                        )
                    st = const.tile([128, N + 2], BF16, name=f"C_{vname}_{h}", tag=f"C_{vname}_{h}")
                    nc.vector.tensor_scalar_add(
                        st[:, 1 : 1 + N], ps[:], b1_s[:, h : h + 1]
                    )
                    c_tiles[(vname, h)] = st

            # ---- border columns j=0 and j=N-1 ----
            for col, rvar in ((0, "left"), (N - 1, "right")):
                hcs = []
                for h in range(2):
                    hc = col_sb.tile([128, ROWS], BF16, tag=f"hc{h}")
                    rfx = r_tiles[(rvar, h)]
                    # bulk: C_mid[col] as per-partition bias
                    nc.scalar.activation(
                        hc[:],
                        rfx[:],
                        ACTF.Relu,
                        bias=c_tiles[("mid", h)][:, col + 1 : col + 2],
                        scale=1.0,
                    )
                    # corners: local rows 0 / ROWS-1 use C_top / C_bot
                    nc.scalar.activation(
                        hc[:, 0:1],
                        rfx[:, 0:1],
                        ACTF.Relu,
                        bias=c_tiles[("top", h)][:, col + 1 : col + 2],
                        scale=1.0,
                    )
                    nc.scalar.activation(
                        hc[:, ROWS - 1 : ROWS],
                        rfx[:, ROWS - 1 : ROWS],
                        ACTF.Relu,
                        bias=c_tiles[("bot", h)][:, col + 1 : col + 2],
                        scale=1.0,
                    )
                    hcs.append(hc)
                pc = col_ps.tile([1, ROWS], F32, tag="pc")
                for h in range(2):
                    nc.tensor.matmul(
                        pc[:],
                        _mm_cast(w2_s[h][:, 0:1], DOT_F32R),
                        _mm_cast(hcs[h][:], DOT_F32R),
                        start=(h == 0),
                        stop=(h == 1),
                    )
                sc = col_sb.tile([1, ROWS], F32, tag="sc")
                nc.vector.tensor_copy(sc[:], pc[:])
                nc.sync.dma_start(
                    y.ap()[0:ROWS, col : col + 1].rearrange("r c -> c r"),
                    sc[:],
                )

            # ---- main loop: 32 groups x 4 rows ----

            ew_acc = 0.0

            def pick_dve():
                nonlocal ew_acc
                ew_acc += DVE_FRAC
                if ew_acc >= 1.0:
                    ew_acc -= 1.0
                    return True
                return False

            group_order = list(range(1, ROWS // 4 - 1)) + [0, ROWS // 4 - 1]
            for g in group_order:
                yp = y_ps.tile([128, N], F32, tag="yp")
                for r in range(4):
                    i = 4 * g + r
                    cvar = "top" if i == 0 else ("bot" if i == ROWS - 1 else "mid")
                    for h in range(2):
                        ht = h_pool.tile([128, NI], BF16, tag=f"h{h}")
                        cv = c_tiles[(cvar, h)]
                        rt = r_tiles[("mid", h)]
                        if pick_dve():
                            nc.vector.tensor_scalar(
                                ht[:],
                                cv[:, 2 : 2 + NI],
                                rt[:, i : i + 1],
                                0.0,
                                ALU.add,
                                ALU.max,
                            )
                        else:
                            nc.scalar.activation(
                                ht[:],
                                cv[:, 2 : 2 + NI],
                                ACTF.Relu,
                                bias=rt[:, i : i + 1],
                                scale=1.0,
                            )
                        nc.tensor.matmul(
                            yp[32 * r : 32 * r + 32, 0:NI],
                            _mm_cast(w2_s[h], DOT_F32R),
                            _mm_cast(ht[:], DOT_F32R),
                            start=(h == 0),
                            stop=(h == 1),
                            tile_position=(0, 32 * r),
                        )
                st = y_st.tile([128, NI], F32, tag="yst")
                if COPY_ALT and g % 2 == 0:
                    nc.vector.tensor_copy(st[:], yp[:, 0:NI])
                else:
                    nc.scalar.copy(st[:], yp[:, 0:NI])
                nc.sync.dma_start(
                    y.ap()[4 * g : 4 * g + 4, 1 : 1 + NI],
                    st[0:128:32, :],
                )

    nc.compile()
    return nc


def _get_nc():
    global _CACHED_NC
    if _CACHED_NC is None:
        _CACHED_NC = _build_program()
    return _CACHED_NC


def _prepare_in_maps(x_l, W1, b1, W2):
    x_l = np.ascontiguousarray(x_l, dtype=np.float32)
    W1 = np.ascontiguousarray(W1, dtype=np.float32)
    W1 = np.concatenate([W1[0:128], W1[128:256], W1[256:384]], axis=1)  # [128, 768]
    b1 = np.ascontiguousarray(b1, dtype=np.float32).reshape(2, 128).T.copy()
    W2 = np.repeat(np.ascontiguousarray(W2, dtype=np.float32).reshape(H, 1), 32, axis=1).astype(np.float16)
    W2 = np.concatenate([W2[0:128], W2[128:256]], axis=1)  # [128, 64]

    in_maps = []
    for k in range(NCORES):
        b = k // (N // ROWS)
        r0 = ROWS * (k % (N // ROWS))
        xT = x_l[b].T  # [CIN, N]
        xTp = np.zeros((CIN, N + 2), np.float32)
        xTp[:, 1 : 1 + N] = xT
        owns_first = r0 == 0
        owns_last = r0 + ROWS == N
        xTq = np.zeros_like(xTp) if owns_first else xTp
        xTr = np.zeros_like(xTp) if owns_last else xTp
        xTh = np.zeros((CIN, ROWS + 2), np.float32)
        lo = max(r0 - 1, 0)
        hi = min(r0 + ROWS + 1, N)
        xTh[:, lo - (r0 - 1) : hi - (r0 - 1)] = xT[:, lo:hi]
        xpqr = np.concatenate([xTp, xTq, xTr], axis=1)
        wpack = np.zeros((128, 132 + 3 * H), np.float32)
        wpack[0:CIN, 0 : ROWS + 2] = xTh
        wpack[:, 130:132] = b1
        wpack[:, 132:] = W1
        in_maps.append(
            {
                "xpqr": np.ascontiguousarray(xpqr),
                "wpack": wpack,
                "w2v": W2,
            }
        )
    return in_maps


def _gather(results, b2):
    yf = np.empty((NCORES * ROWS, N), np.float32)
    for k in range(NCORES):
        yf[ROWS * k : ROWS * (k + 1)] = results[k]["y"]
    yf += np.float32(b2.reshape(-1)[0])
    return yf.reshape(B, N, N, 1)


def kernel(x_l, W1, b1, W2, b2, trace=False):
    global LAST_RESULTS
    nc = _get_nc()
    in_maps = _prepare_in_maps(x_l, W1, b1, W2)
    try:
        res = bass_utils.run_bass_kernel_spmd(
            nc, in_maps, core_ids=list(range(NCORES)), trace=trace
        )
    except Exception:
        # transient device-unrecoverable states have been observed to clear
        # on retry; give it one more attempt before failing
        res = bass_utils.run_bass_kernel_spmd(
            nc, in_maps, core_ids=list(range(NCORES)), trace=trace
        )
    LAST_RESULTS = res
    return _gather(res.results, np.asarray(b2, dtype=np.float32))

